# revision 9
# baseline (speedup 1.0000x reference)
"""Trainium2 Bass kernel for nn_AttentionBlock (B=16, C=256, H=W=32, NH=4, GROUPS=8).

Strategy: data-parallel over batch. 8 cores x 2 batch elements each; no
collectives. Per batch element, everything is kept in [channels, spatial]
layout (channels on SBUF partitions):

  1. GroupNorm: per-channel sum/sumsq on DVE (free-dim reduce), group
     aggregation via a tiny matmul against a block-diagonal averaging matrix,
     rstd computed on DVE via the magic-constant rsqrt seed + 2 Newton
     iterations (keeps ACT exclusively on the exp table -> no table swaps).
  2. qkv 1x1 conv: Q,K produced as [o, s] tiles bf16 (weights stationary);
     V produced directly TRANSPOSED as v^T [s, d-block] tiles (hn stationary)
     and quantized to fp8e4 at evacuation.
  3. Attention per head (d=64): scores computed transposed,
     S^T[k, q] = K_dS^T . Q_dS bf16, with two heads packed into the PE array
     via 64x128 row tiling. exp on ACT reads PSUM directly and writes
     fp8e4 expS^T (scale=1/8 and a -2 shift folded in; the shift cancels in
     softmax and keeps everything in fp8 range). P@V and the denominator
     matmuls run in fp8 with DoubleRow perf mode (2 k-planes per pass).
  4. Normalize with DVE reciprocal_approx_fast + DVE mult -> fp8 at tiles.
  5. proj 1x1 conv in fp8 DoubleRow + residual + bias fused into the
     PSUM->SBUF evacuation.
"""

import sys

sys.path.insert(0, "/opt/trn_rl_repo")

from contextlib import ExitStack

import numpy as np
import ml_dtypes

import concourse.bass as bass
import concourse.tile as tile
from concourse import bacc, mybir
from concourse.bass_utils import run_bass_kernel_spmd

F32 = mybir.dt.float32
BF16 = mybir.dt.bfloat16
FP8 = mybir.dt.float8e4
I32 = mybir.dt.int32
AF = mybir.ActivationFunctionType
OP = mybir.AluOpType
DR = mybir.MatmulPerfMode.DoubleRow

N_CORES = 8
B_PER = 2          # batch elements per core
C = 256
S = 1024           # H*W
NH = 4
D = 64             # head dim
EPS = 1e-5
CT = C // 128      # channel tiles (2)
KT = S // 128      # key/s tiles (8)
QC = S // 512      # q chunks of 512 (2)
SHIFT = -2.0       # exp(s/8 + SHIFT); cancels in softmax, keeps fp8 in range
MAGIC = 0x5F3759DF


def build_nc():
    nc = bacc.Bacc("TRN2", target_bir_lowering=False, debug=False,
                   num_devices=N_CORES)

    x_d = nc.dram_tensor("x", [B_PER, C, S], F32, kind="ExternalInput").ap()
    wqkvT_d = nc.dram_tensor("wqkvT", [C, 3 * C], BF16, kind="ExternalInput").ap()
    wprojT_d = nc.dram_tensor("wprojT", [128, 2 * C], FP8, kind="ExternalInput").ap()
    qkb_d = nc.dram_tensor("qkb", [128, 4], F32, kind="ExternalInput").ap()
    bv_d = nc.dram_tensor("bv", [128, C], F32, kind="ExternalInput").ap()
    pb_d = nc.dram_tensor("pb", [128, 2], F32, kind="ExternalInput").ap()
    nw_d = nc.dram_tensor("nw", [128, 2], F32, kind="ExternalInput").ap()
    nb_d = nc.dram_tensor("nb", [128, 2], F32, kind="ExternalInput").ap()
    g_d = nc.dram_tensor("G", [128, 128], F32, kind="ExternalInput").ap()
    out_d = nc.dram_tensor("out", [B_PER, C, S], F32, kind="ExternalOutput").ap()

    with tile.TileContext(nc) as tc, ExitStack() as ctx:
        # ---- pools (bufs is per-tag) ----
        cpool = ctx.enter_context(tc.tile_pool(name="consts", bufs=1))
        xpool = ctx.enter_context(tc.tile_pool(name="x", bufs=1))
        hnpool = ctx.enter_context(tc.tile_pool(name="hn", bufs=1))
        qkpool = ctx.enter_context(tc.tile_pool(name="qk", bufs=1))
        vtpool = ctx.enter_context(tc.tile_pool(name="vt", bufs=1))
        expool = ctx.enter_context(tc.tile_pool(name="expS", bufs=1))
        atpool = ctx.enter_context(tc.tile_pool(name="attn", bufs=1))
        bcpool = ctx.enter_context(tc.tile_pool(name="bcast", bufs=2))
        opool = ctx.enter_context(tc.tile_pool(name="osb", bufs=2))
        scpool = ctx.enter_context(tc.tile_pool(name="scratch", bufs=1))
        vecpool = ctx.enter_context(tc.tile_pool(name="vec", bufs=2))

        ps_scores = ctx.enter_context(tc.tile_pool(name="ps_sc", bufs=2,
                                                   space="PSUM"))
        ps_attn = ctx.enter_context(tc.tile_pool(name="ps_at", bufs=1,
                                                 space="PSUM"))
        ps_qkv = ctx.enter_context(tc.tile_pool(name="ps_qkv", bufs=2,
                                                space="PSUM"))

        # ---- constants ----
        wq = [cpool.tile([128, 3 * C], BF16, name=f"wq{i}", tag=f"wq{i}")
              for i in range(CT)]
        for i in range(CT):
            nc.sync.dma_start(wq[i][:], wqkvT_d[128 * i:128 * (i + 1), :])
        wp = cpool.tile([128, 2, C], FP8, name="wp", tag="wp")
        nc.sync.dma_start(wp[:], wprojT_d[:])
        qkb = cpool.tile([128, 4], F32, name="qkb", tag="qkb")
        nc.sync.dma_start(qkb[:], qkb_d[:])
        bv = cpool.tile([128, C], F32, name="bv", tag="bv")
        nc.sync.dma_start(bv[:], bv_d[:])
        pb = cpool.tile([128, 2], F32, name="pb", tag="pb")
        nc.sync.dma_start(pb[:], pb_d[:])
        nw = cpool.tile([128, 2], F32, name="nw", tag="nw")
        nc.sync.dma_start(nw[:], nw_d[:])
        nb = cpool.tile([128, 2], F32, name="nb", tag="nb")
        nc.sync.dma_start(nb[:], nb_d[:])
        G = cpool.tile([128, 128], F32, name="G", tag="G")
        nc.sync.dma_start(G[:], g_d[:])
        denw = cpool.tile([128, 2, D], FP8, name="denw", tag="denw")
        nc.vector.memset(denw[:], 1.0)
        magic = cpool.tile([128, 2], I32, name="magic", tag="magic")
        nc.vector.memset(magic[:], MAGIC)
        shiftc = cpool.tile([128, 1], F32, name="shiftc", tag="shiftc")
        nc.vector.memset(shiftc[:], SHIFT)

        # per-batch state
        xt = {}      # (b, ct) -> x tile [128, 1024] f32
        hnt = {}     # (b, ct) -> hn tile [128, 1024] bf16
        qkt = {}     # (b, j) -> j in 0..3: Q m-tiles 0,1; K m-tiles 2,3
        vtt = {}     # b -> v^T tile [128, KT, 256] fp8 (t-planes, head h at 64h)
        expt = {}    # (pair, a) -> expS^T tile [128, KT, 1024] fp8 (t-planes)
        att = {}     # b -> at tile [128, 2, 1024] fp8 (hp planes)

        scratch = scpool.tile([128, 1024], F32, name="scr", tag="scr")

        def emit_gn(b):
            """GroupNorm stats + apply for batch b (DVE + tiny PE matmul)."""
            stats = vecpool.tile([128, 4], F32, name=f"st{b}", tag="stats")
            nvar = vecpool.tile([128, 2], F32, name=f"nv{b}", tag="nvar")
            veps = vecpool.tile([128, 2], F32, name=f"ve{b}", tag="veps")
            yis = vecpool.tile([128, 2], I32, name=f"yi{b}", tag="yis")
            rstd = vecpool.tile([128, 2], F32, name=f"rs{b}", tag="rstd")
            hneg = vecpool.tile([128, 2], F32, name=f"hg{b}", tag="hneg")
            tsq = vecpool.tile([128, 2], F32, name=f"tq{b}", tag="tsq")
            usq = vecpool.tile([128, 2], F32, name=f"uq{b}", tag="usq")
            Av = vecpool.tile([128, 2], F32, name=f"A{b}", tag="Av")
            nBv = vecpool.tile([128, 2], F32, name=f"nB{b}", tag="nBv")
            gsb = vecpool.tile([128, 4], F32, name=f"gs{b}", tag="gsb")
            for ct in range(CT):
                xtile = xpool.tile([128, 1024], F32, name=f"x{b}{ct}",
                                   tag=f"x{b}{ct}")
                nc.sync.dma_start(xtile[:], x_d[b, 128 * ct:128 * (ct + 1), :])
                xt[(b, ct)] = xtile
                nc.vector.tensor_reduce(
                    out=stats[:, ct:ct + 1], in_=xtile[:],
                    axis=mybir.AxisListType.X, op=OP.add)
                nc.vector.scalar_tensor_tensor(
                    out=scratch[:], in0=xtile[:], scalar=1.0, in1=xtile[:],
                    op0=OP.bypass, op1=OP.mult,
                    accum_out=stats[:, 2 + ct:3 + ct])
            # group-average via G matmul: gps = [mean0, mean1, E2_0, E2_1]
            gps = ps_qkv.tile([128, 4], F32, name=f"g{b}", tag="qkv")
            nc.tensor.matmul(out=gps[:], lhsT=G[:], rhs=stats[:],
                             start=True, stop=True)
            nc.vector.tensor_copy(gsb[:], gps[:])
            means = gsb[:, 0:2]   # [128, 2] means for ct 0/1
            e2s = gsb[:, 2:4]
            # nvar = mean^2 - E2 ; veps = -nvar + eps = var + eps
            nc.vector.tensor_tensor(out=nvar[:], in0=means, in1=means,
                                    op=OP.mult)
            nc.vector.tensor_tensor(out=nvar[:], in0=nvar[:], in1=e2s,
                                    op=OP.subtract)
            nc.vector.tensor_scalar(
                out=veps[:], in0=nvar[:], scalar1=-1.0, scalar2=EPS,
                op0=OP.mult, op1=OP.add)
            # rstd = rsqrt(veps): magic seed + 2 Newton iterations
            nc.vector.tensor_scalar(
                out=yis[:], in0=veps[:].bitcast(I32), scalar1=1, scalar2=None,
                op0=OP.arith_shift_right)
            nc.vector.tensor_tensor(
                out=yis[:], in0=magic[:], in1=yis[:], op=OP.subtract)
            y = yis[:].bitcast(F32)
            nc.vector.tensor_scalar(
                out=hneg[:], in0=veps[:], scalar1=-0.5, scalar2=None,
                op0=OP.mult)
            for it in range(2):
                dst = rstd[:] if it == 1 else y
                nc.vector.tensor_tensor(out=tsq[:], in0=y, in1=y, op=OP.mult)
                nc.vector.tensor_tensor(out=usq[:], in0=tsq[:], in1=hneg[:],
                                        op=OP.mult)
                nc.vector.scalar_tensor_tensor(
                    out=dst, in0=usq[:], scalar=1.5, in1=y,
                    op0=OP.add, op1=OP.mult)
            # A = rstd * nw ; negB = mean*A - nb   (hn = x*A - negB)
            nc.vector.tensor_mul(Av[:], rstd[:], nw[:])
            nc.vector.tensor_tensor(out=nBv[:], in0=means, in1=Av[:],
                                    op=OP.mult)
            nc.vector.tensor_tensor(out=nBv[:], in0=nBv[:], in1=nb[:],
                                    op=OP.subtract)
            for ct in range(CT):
                hn = hnpool.tile([128, 1024], BF16, name=f"hn{b}{ct}",
                                 tag=f"hn{b}{ct}")
                nc.vector.tensor_scalar(
                    out=hn[:], in0=xt[(b, ct)][:], scalar1=Av[:, ct:ct + 1],
                    scalar2=nBv[:, ct:ct + 1], op0=OP.mult, op1=OP.subtract)
                hnt[(b, ct)] = hn

        def emit_qkv(b):
            """Q,K as [o,s] bf16 tiles; V transposed as v^T [s,d] fp8 planes."""
            for j in range(4):
                qk = qkpool.tile([128, 1024], BF16, name=f"qk{b}{j}",
                                 tag=f"qk{b}{j}")
                for qc in range(QC):
                    ps = ps_qkv.tile([128, 512], F32, name=f"qp{b}{j}{qc}",
                                     tag="qkv")
                    for k in range(CT):
                        nc.tensor.matmul(
                            out=ps[:],
                            lhsT=wq[k][:, 128 * j:128 * (j + 1)],
                            rhs=hnt[(b, k)][:, 512 * qc:512 * (qc + 1)],
                            start=(k == 0), stop=(k == CT - 1))
                    nc.vector.tensor_scalar(
                        out=qk[:, 512 * qc:512 * (qc + 1)], in0=ps[:],
                        scalar1=qkb[:, j:j + 1], scalar2=None, op0=OP.add)
                qkt[(b, j)] = qk
            # V^T: s-tile planes, [128 (s), KT, 256 (4 heads x 64)] fp8
            vt = vtpool.tile([128, KT, 256], FP8, name=f"vt{b}", tag=f"vt{b}")
            for t in range(KT):
                ps = ps_qkv.tile([128, 256], F32, name=f"vp{b}{t}", tag="qkv")
                for k in range(CT):
                    nc.tensor.matmul(
                        out=ps[:],
                        lhsT=hnt[(b, k)][:, 128 * t:128 * (t + 1)],
                        rhs=wq[k][:, 512:768],
                        start=(k == 0), stop=(k == CT - 1))
                nc.vector.scalar_tensor_tensor(
                    out=vt[:, t, :], in0=ps[:], scalar=1.0,
                    in1=bv[:], op0=OP.bypass, op1=OP.add)
            vtt[b] = vt

        def emit_scores(p):
            """mm1 + exp for pair p: batch p//2, heads (0,1) or (2,3)."""
            b, hp = divmod(p, 2)
            qA = qkt[(b, hp)]      # Q m-tile hp: head 2hp rows 0-63, 2hp+1 rows 64-127
            kA = qkt[(b, 2 + hp)]  # K m-tile
            eA = expool.tile([128, KT, 1024], FP8, name=f"ex{p}a", tag=f"ex{p % 2}a")
            eB = expool.tile([128, KT, 1024], FP8, name=f"ex{p}b", tag=f"ex{p % 2}b")
            expt[(p, 0)], expt[(p, 1)] = eA, eB
            for t in range(KT):
                chA = ps_scores.tile([128, 1024], F32, name=f"sA{p}{t}", tag="sc")
                chB = ps_scores.tile([128, 1024], F32, name=f"sB{p}{t}", tag="sc")
                for qc in range(QC):
                    nc.tensor.matmul(
                        out=chA[:, 512 * qc:512 * (qc + 1)],
                        lhsT=kA[0:64, 128 * t:128 * (t + 1)],
                        rhs=qA[0:64, 512 * qc:512 * (qc + 1)],
                        start=True, stop=True, tile_position=(0, 0))
                    nc.tensor.matmul(
                        out=chB[:, 512 * qc:512 * (qc + 1)],
                        lhsT=kA[64:128, 128 * t:128 * (t + 1)],
                        rhs=qA[64:128, 512 * qc:512 * (qc + 1)],
                        start=True, stop=True, tile_position=(64, 0))
                nc.scalar.activation(eA[:, t, :], chA[:], AF.Exp,
                                     bias=shiftc[:, 0:1], scale=0.125)
                nc.scalar.activation(eB[:, t, :], chB[:], AF.Exp,
                                     bias=shiftc[:, 0:1], scale=0.125)

        def emit_mm2den(p):
            """P@V + denominators (fp8 DoubleRow) + normalize."""
            b, hp = divmod(p, 2)
            eA, eB = expt[(p, 0)], expt[(p, 1)]
            vt = vtt[b]
            hA, hB = 2 * hp, 2 * hp + 1
            if hp == 0:
                at = atpool.tile([128, 2, 1024], FP8, name=f"at{b}", tag=f"at{b}")
                att[b] = at
            at = att[b]
            u = ps_attn.tile([128, 1024], F32, name=f"u{p}", tag="at")
            for qc in range(QC):
                for tp in range(0, KT, 2):
                    # DoubleRow dst must start at partition 0 (ISA quadrant
                    # rule), so only head A gets DR; head B runs plain fp8.
                    nc.tensor.matmul(
                        out=u[0:64, 512 * qc:512 * (qc + 1)],
                        lhsT=vt[:, tp:tp + 2, 64 * hA:64 * hA + 64],
                        rhs=eA[:, tp:tp + 2, 512 * qc:512 * (qc + 1)],
                        start=(tp == 0), stop=(tp == KT - 2),
                        perf_mode=DR,
                        tile_position=(0, 0), skip_group_check=True)
                for t in range(KT):
                    nc.tensor.matmul(
                        out=u[64:128, 512 * qc:512 * (qc + 1)],
                        lhsT=vt[:, t, 64 * hB:64 * hB + 64],
                        rhs=eB[:, t, 512 * qc:512 * (qc + 1)],
                        start=(t == 0), stop=(t == KT - 1),
                        tile_position=(0, 64), skip_group_check=True)
            # denominators (fp8 DoubleRow against all-ones stationary): each
            # den tile holds the denominator replicated across partitions
            # 0-63 (head A) / 64-127 (head B).
            rc = bcpool.tile([128, 1024], F32, name=f"rc{p}", tag="rc")
            for qc in range(QC):
                den = ps_qkv.tile([128, 512], F32, name=f"dn{p}{qc}", tag="qkv")
                for tp in range(0, KT, 2):
                    nc.tensor.matmul(
                        out=den[0:64, :],
                        lhsT=denw[:],
                        rhs=eA[:, tp:tp + 2, 512 * qc:512 * (qc + 1)],
                        start=(tp == 0), stop=(tp == KT - 2),
                        perf_mode=DR,
                        tile_position=(0, 0), skip_group_check=True)
                for t in range(KT):
                    nc.tensor.matmul(
                        out=den[64:128, :],
                        lhsT=denw[:, 0, :],
                        rhs=eB[:, t, 512 * qc:512 * (qc + 1)],
                        start=(t == 0), stop=(t == KT - 1),
                        tile_position=(0, 64), skip_group_check=True)
                nc.vector.reciprocal_approx_fast(
                    rc[:, 512 * qc:512 * (qc + 1)], den[:])
            nc.vector.tensor_mul(at[:, hp, :], u[:], rc[:])

        def emit_proj(b):
            """proj (fp8 DoubleRow) + residual + bias, then store."""
            at = att[b]
            for m in range(CT):
                ps = ps_attn.tile([128, 1024], F32, name=f"pj{b}{m}", tag="at")
                for qc in range(QC):
                    nc.tensor.matmul(
                        out=ps[:, 512 * qc:512 * (qc + 1)],
                        lhsT=wp[:, :, 128 * m:128 * (m + 1)],
                        rhs=at[:, :, 512 * qc:512 * (qc + 1)],
                        start=True, stop=True, perf_mode=DR)
                osb = opool.tile([128, 1024], F32, name=f"o{b}{m}", tag="osb")
                nc.vector.scalar_tensor_tensor(
                    out=osb[:], in0=ps[:], scalar=pb[:, m:m + 1],
                    in1=xt[(b, m)][:], op0=OP.add, op1=OP.add)
                nc.sync.dma_start(out_d[b, 128 * m:128 * (m + 1), :], osb[:])

        # ---- software-pipelined emission ----
        emit_gn(0)
        emit_qkv(0)
        emit_gn(1)
        emit_scores(0)
        emit_qkv(1)
        emit_scores(1)
        emit_mm2den(0)
        emit_scores(2)
        emit_mm2den(1)
        emit_proj(0)
        emit_scores(3)
        emit_mm2den(2)
        emit_mm2den(3)
        emit_proj(1)

    nc.compile()
    return nc


_NC = None


def _get_nc():
    global _NC
    if _NC is None:
        _NC = build_nc()
    return _NC


def make_in_maps(x, norm_w, norm_b, qkv_w, qkv_b, proj_w, proj_b):
    x = np.asarray(x, dtype=np.float32)
    B = x.shape[0]
    assert B == N_CORES * B_PER

    wqkvT = np.ascontiguousarray(np.asarray(qkv_w, np.float32).T).astype(
        ml_dtypes.bfloat16)  # [C, 3C]
    # proj weights as [128, 2 (ct plane), C] fp8 for DoubleRow
    wpT = np.ascontiguousarray(np.asarray(proj_w, np.float32).T)  # [C(in), C(out)]
    wprojT = np.ascontiguousarray(
        wpT.reshape(2, 128, C).transpose(1, 0, 2).reshape(128, 2 * C)
    ).astype(ml_dtypes.float8_e4m3)
    qkb = np.ascontiguousarray(
        np.asarray(qkv_b[:512], np.float32).reshape(4, 128).T)  # [128, 4]
    bv = np.broadcast_to(np.asarray(qkv_b[512:768], np.float32),
                         (128, C)).copy()
    pb = np.ascontiguousarray(np.asarray(proj_b, np.float32).reshape(2, 128).T)
    nw = np.ascontiguousarray(np.asarray(norm_w, np.float32).reshape(2, 128).T)
    nb = np.ascontiguousarray(np.asarray(norm_b, np.float32).reshape(2, 128).T)
    # block-diagonal group-average matrix, 1/(32*1024) normalizer folded in
    G = np.zeros((128, 128), np.float32)
    for g in range(4):
        G[32 * g:32 * (g + 1), 32 * g:32 * (g + 1)] = 1.0 / (32.0 * 1024.0)

    xs = x.reshape(N_CORES, B_PER, C, S)
    common = dict(wqkvT=wqkvT, wprojT=wprojT, qkb=qkb, bv=bv, pb=pb, nw=nw,
                  nb=nb, G=G)
    return [dict(x=np.ascontiguousarray(xs[i]), **common)
            for i in range(N_CORES)]


def kernel(x, norm_w, norm_b, qkv_w, qkv_b, proj_w, proj_b):
    in_maps = make_in_maps(x, norm_w, norm_b, qkv_w, qkv_b, proj_w, proj_b)
    nc = _get_nc()
    res = run_bass_kernel_spmd(nc, in_maps, core_ids=list(range(N_CORES)))
    out = np.stack([res.results[i]["out"] for i in range(N_CORES)], axis=0)
    return out.reshape(x.shape[0], C, 32, 32).astype(np.float32)
